# revision 32
# baseline (speedup 1.0000x reference)
"""CVRP decoder kernel for Trainium2 (8 NeuronCores, batch-data-parallel).

Computes, per batch b (B=64, P=64, N=1000, H=128):
    q_graph   = mean_n(emb) @ Wq_graph
    q_first   = encoded_q1 @ Wq_first
    q_last    = emb[last_node] @ Wq_last
    q_visited = (vis01 @ emb / N) @ W_visited          (vis01 = isneginf(mask))
    final_q   = sum of the above + load*W_load + b_load
    score     = final_q @ emb^T / sqrt(H) - dists[last_node] / sqrt(2)
    probs     = softmax(10*tanh(score) + (-BIG if visited))

Sharding: batch dim across the 8 cores (pure data parallel), 8 batches per
core processed as 4 pairs of 2 batches stacked on the 128 SBUF partitions.

Key structure:
- all input DMAs staged first (emb with 4KB descriptors via the n = 8p+c
  interleave; the column permutation rides through the pipeline and is
  undone for free by the strided write of the final normalize)
- compute split into phase A (cast/transpose/qv/final_q for all pairs) and
  phase B (score + softmax per pair) so each engine queue runs dense
  bursts instead of stalling at cross-engine dependencies
- all matmuls bf16 (1 PE cycle/row); gathered dist rows enter the score
  psum via an fp16 identity matmul, with fqT negated so tanh's oddness
  recovers the sign: probs = exp(-10*(t' - mb)) / sum
"""

import json
import math
import numpy as np
from contextlib import ExitStack

import concourse.bass as bass
import concourse.mybir as mybir
import concourse.tile as tile
from concourse.bass_utils import run_bass_kernel_spmd
from concourse.masks import make_identity


def _split_excess_waits(bir_bytes: bytes, max_waits: int = 1) -> bytes:
    """Walrus in this image rejects instructions carrying too many sem waits
    ("Too many sync wait commands", e.g. on Tile's kernel-tail Drain).
    Hoist excess waits onto preceding same-engine EventSemaphore carriers
    (pure sync ops) — sems are monotonic, so a chain of instructions whose
    waits partition the original list is equivalent."""
    d = json.loads(bir_bytes)
    n = [0]
    for fn in d.get("functions", []):
        for blk in fn.get("blocks", []):
            out = []
            for ins in blk.get("instructions", []):
                si = ins.get("sync_info") or {}
                waits = si.get("on_wait") or []
                if len(waits) > max_waits:
                    extra, keep = waits[:-max_waits], waits[-max_waits:]
                    ins["sync_info"]["on_wait"] = keep
                    for i in range(0, len(extra), max_waits):
                        n[0] += 1
                        carrier = {
                            "name": f"I-waitsplit-{n[0]}",
                            "opcode": "EventSemaphore",
                            "engine": ins["engine"],
                            "ins": [],
                            "outs": [],
                            "sync_info": {
                                "on_update": [],
                                "on_wait": extra[i:i + max_waits],
                            },
                        }
                        if "debug" in ins:
                            carrier["debug"] = ins["debug"]
                        out.append(carrier)
                out.append(ins)
            blk["instructions"] = out
    return json.dumps(d).encode()


def _install_walrus_shim():
    import concourse.bass2jax as b2j
    import concourse.bass_utils as bu
    if getattr(bu, "_waitsplit_installed", False):
        return
    real = bu.compile_bir_kernel

    def patched(bir_json, tmpdir, neff_name="file.neff", **kw):
        if isinstance(bir_json, (bytes, bytearray, str)):
            if isinstance(bir_json, str):
                bir_json = bir_json.encode()
            bir_json = _split_excess_waits(bir_json)
        return real(bir_json, tmpdir, neff_name=neff_name, **kw)

    bu.compile_bir_kernel = patched
    b2j.compile_bir_kernel = patched
    bu._waitsplit_installed = True


_install_walrus_shim()

F32 = mybir.dt.float32
FP16 = mybir.dt.float16
BF16 = mybir.dt.bfloat16
I32 = mybir.dt.int32
OP = mybir.AluOpType
AF = mybir.ActivationFunctionType

B, P, N, H = 64, 64, 1000, 128
NCORES = 8
NB = B // NCORES          # 8 batches per core
NPAIR = NB // 2           # 4 pairs
NC = 8                    # n-chunks; n = 8p + c with p < 125 (4KB DMA runs)
NP = 125                  # rows per chunk; perm col = 125c + p

MASK_NEG = -1000.0        # additive bias for visited nodes (pre x10 exp scale)
QV_SCALE = -1.0 / (1000.0 * N)   # undo MASK_NEG and the /N in one eviction
FQ_SCALE = math.sqrt(2.0) / math.sqrt(H)   # = 0.125 exactly
TANH_SCALE = 1.0 / math.sqrt(2.0)
TANH_CLIP = 10.0


def build_nc():
    nc = bass.Bass()

    dists = nc.dram_tensor("dists", [NB * N, N], F32, kind="ExternalInput")
    emb = nc.dram_tensor("emb", [NB * N, H], F32, kind="ExternalInput")
    eq1 = nc.dram_tensor("eq1", [NB * P, H], F32, kind="ExternalInput")
    lastnode = nc.dram_tensor("lastnode", [NB * P, 1], I32, kind="ExternalInput")
    loadv = nc.dram_tensor("loadv", [1, NB * P], F32, kind="ExternalInput")
    maskt = nc.dram_tensor("maskt", [NB * P, N], F32, kind="ExternalInput")
    wq_graph = nc.dram_tensor("wq_graph", [H, H], F32, kind="ExternalInput")
    wq_first = nc.dram_tensor("wq_first", [H, H], F32, kind="ExternalInput")
    wq_last = nc.dram_tensor("wq_last", [H, H], F32, kind="ExternalInput")
    w_visited = nc.dram_tensor("w_visited", [H, H], F32, kind="ExternalInput")
    w_load = nc.dram_tensor("w_load", [1, H], F32, kind="ExternalInput")
    b_load = nc.dram_tensor("b_load", [1, H], F32, kind="ExternalInput")
    probs = nc.dram_tensor("probs", [NB * P, N], F32, kind="ExternalOutput")

    with tile.TileContext(nc) as tc:
        with ExitStack() as ctx:
            const = ctx.enter_context(tc.tile_pool(name="const", bufs=1))
            stg = ctx.enter_context(tc.tile_pool(name="stg", bufs=NPAIR))
            sb4 = ctx.enter_context(tc.tile_pool(name="sb4", bufs=NPAIR))
            sb = ctx.enter_context(tc.tile_pool(name="sb", bufs=2))
            sbc = ctx.enter_context(tc.tile_pool(name="sbc", bufs=2))
            sbe = ctx.enter_context(tc.tile_pool(name="sbe", bufs=4))
            ps_tr = ctx.enter_context(
                tc.tile_pool(name="ps_tr", bufs=2, space="PSUM"))
            ps_sm = ctx.enter_context(
                tc.tile_pool(name="ps_sm", bufs=2, space="PSUM"))
            ps_sc = ctx.enter_context(
                tc.tile_pool(name="ps_sc", bufs=2, space="PSUM"))

            # ---- tiny constants needed by staging/compute ----
            identb = const.tile([128, 128], BF16, tag="identb")
            make_identity(nc, identb[:])
            identf = const.tile([128, 128], F32, tag="identf")
            make_identity(nc, identf[:])
            identh = const.tile([128, 128], FP16, tag="identh")
            make_identity(nc, identh[:])
            adj_all = const.tile([128, NPAIR], I32, tag="adj_all")
            for pr in range(NPAIR):
                nc.gpsimd.memset(adj_all[0:64, pr:pr + 1], N * 2 * pr)
                nc.gpsimd.memset(adj_all[64:128, pr:pr + 1], N * (2 * pr + 1))

            # ---- stage all per-pair input DMAs (sync queue) ----
            S = {}
            for pr in range(NPAIR):
                b0 = 2 * pr
                r0 = 128 * pr
                idxr = stg.tile([128, 1], I32, tag="idxr")
                nc.sync.dma_start(idxr[:], lastnode[r0:r0 + 128, :])
                idxa = stg.tile([128, 1], I32, tag="idxa")
                nc.vector.tensor_tensor(out=idxa[:], in0=idxr[:],
                                        in1=adj_all[:, pr:pr + 1], op=OP.add)
                mk = stg.tile([128, N], F32, tag="mk")
                nc.sync.dma_start(mk[:], maskt[r0:r0 + 128, :])
                emb_n = []
                for j in range(2):
                    base = (b0 + j) * N
                    # 4KB descriptors: partition p holds rows 8p..8p+7
                    e = stg.tile([NP, NC, H], F32, tag=f"embn{j}")
                    nc.sync.dma_start(
                        e[:], emb[base:base + N, :]
                        .rearrange("(p c) h -> p (c h)", p=NP))
                    emb_n.append(e)
                lastemb = stg.tile([128, H], F32, tag="lastemb")
                nc.gpsimd.indirect_dma_start(
                    out=lastemb[:], out_offset=None, in_=emb[:],
                    in_offset=bass.IndirectOffsetOnAxis(ap=idxa[:, 0:1], axis=0))
                distg = stg.tile([128, N], F32, tag="distg")
                nc.gpsimd.indirect_dma_start(
                    out=distg[:], out_offset=None, in_=dists[:],
                    in_offset=bass.IndirectOffsetOnAxis(ap=idxa[:, 0:1], axis=0))
                S[pr] = dict(idxa=idxa, mk=mk, emb_n=emb_n, lastemb=lastemb,
                             distg=distg)

            # ---- remaining constants (scalar queue; overlap with staging)
            wtiles = {}
            for name, dram in (("wg", wq_graph), ("wf", wq_first),
                               ("wl", wq_last), ("wv", w_visited)):
                wf32 = const.tile([H, H], F32, tag=name + "f")
                nc.scalar.dma_start(wf32[:], dram[:])
                wb = const.tile([H, H], BF16, tag=name)
                nc.vector.tensor_copy(out=wb[:], in_=wf32[:])
                wtiles[name] = wb
            wldf = const.tile([2, H], F32, tag="wldf")
            nc.scalar.dma_start(wldf[0:1, :], w_load[:])
            nc.scalar.dma_start(wldf[1:2, :], b_load[:])
            wldb = const.tile([2, H], BF16, tag="wldb")
            nc.vector.tensor_copy(out=wldb[:], in_=wldf[:])
            ldf = const.tile([1, NB * P], F32, tag="ldf")
            nc.scalar.dma_start(ldf[:], loadv[:])
            ld2 = const.tile([2, NB * P], BF16, tag="ld2")
            nc.gpsimd.memset(ld2[:], 1.0)
            nc.vector.tensor_copy(out=ld2[0:1, :], in_=ldf[:])

            # eq1^T for all pairs, done once: eq1T_all[h, 128*pr + q]
            eq1a = const.tile([128, NPAIR, H], F32, tag="eq1a")
            nc.scalar.dma_start(
                eq1a[:], eq1[:].rearrange("(r q) h -> q r h", q=128))
            eq1T_all = const.tile([128, NPAIR * 128], BF16, tag="eq1T_all")
            for pr in range(NPAIR):
                pse = ps_sm.tile([128, 128], F32, tag="sm")
                nc.tensor.transpose(out=pse[:], in_=eq1a[:, pr, :],
                                    identity=identf[:])
                nc.vector.tensor_copy(
                    out=eq1T_all[:, 128 * pr:128 * (pr + 1)], in_=pse[:])

            # ---- phase A: casts, transposes, q_visited, final_q ----
            A = {}
            for pr in range(NPAIR):
                r0 = 128 * pr
                mk = S[pr]["mk"]
                emb_n = S[pr]["emb_n"]

                # maskbias = (mask < -1e30) * (-1000) in {0,-1000} bf16,
                # permuted: mb[q, 125c+p] = bias(mask[q, 8p+c])
                mb = sb4.tile([128, N], BF16, tag="mb")
                nc.vector.tensor_scalar(
                    out=mb[:].rearrange("q (c p) -> q c p", c=NC),
                    in0=mk[:].rearrange("q (p c) -> q c p", c=NC),
                    scalar1=-1e30, scalar2=MASK_NEG,
                    op0=OP.is_lt, op1=OP.mult)

                # fp16 dist rows (permuted), for the score-psum injection
                dist16 = sb4.tile([128, N], FP16, tag="dist16")
                nc.vector.tensor_copy(
                    out=dist16[:].rearrange("q (c p) -> q c p", c=NC),
                    in_=S[pr]["distg"][:].rearrange("q (p c) -> q c p", c=NC))

                # emb -> bf16 (split across scalar/vector)
                emb_bf = []
                for j in range(2):
                    eb = sbe.tile([NP, NC, H], BF16, tag="embbf")
                    if j == 0:
                        nc.vector.tensor_copy(out=eb[:], in_=emb_n[j][:])
                    else:
                        nc.scalar.copy(eb[:], emb_n[j][:])
                    emb_bf.append(eb)

                # embT per batch: embT[h, 125c+p] = emb[8p+c, h] bf16
                embT = []
                maccs = []
                for j in range(2):
                    pT = ps_tr.tile([128, NC, 126], BF16, tag="tr")
                    for c in range(NC):
                        nc.tensor.transpose(
                            out=pT[:, c, 0:NP], in_=emb_bf[j][:, c, :],
                            identity=identb[0:NP, 0:NP])
                    et = sb4.tile([128, N], BF16, tag=f"embT{j}")
                    acc = sb.tile([128, 1], F32, tag="macc")
                    nc.scalar.activation(
                        et[:].rearrange("h (c p) -> h c p", c=NC),
                        pT[:, :, 0:NP], AF.Copy, accum_out=acc[:])
                    embT.append(et)
                    maccs.append(acc)

                # mbT[p, c, q] = mb[q, 125c+p] = bias(mask[q, 8p+c])
                pM = ps_tr.tile([NP, NC, 128], BF16, tag="tr")
                for c in range(NC):
                    nc.tensor.transpose(
                        out=pM[:, c, :], in_=mb[:, NP * c:NP * (c + 1)],
                        identity=identb[:])
                mbT = sb.tile([NP, NC, 128], BF16, tag="mbT")
                nc.vector.tensor_copy(out=mbT[:], in_=pM[:])

                # mean broadcast over the p dim: [128, 128] bf16
                meanrep = sb.tile([128, 128], BF16, tag="meanrep")
                for j in range(2):
                    nc.vector.tensor_scalar(
                        out=meanrep[:, 64 * j:64 * j + 64],
                        in0=maccs[j][:, 0:1].to_broadcast([128, 64]),
                        scalar1=1.0 / N, scalar2=None, op0=OP.mult)

                # last-node embedding transpose
                ps_t2 = ps_sm.tile([128, 128], F32, tag="sm")
                nc.tensor.transpose(out=ps_t2[:], in_=S[pr]["lastemb"][:],
                                    identity=identf[:])
                lastembT = sb.tile([128, 128], BF16, tag="lastembT")
                nc.vector.tensor_copy(out=lastembT[:], in_=ps_t2[:])

                # q_visited pre: psum[h, j*64+p] = sum_n emb[n,h]*mbias[q,n]
                psq = ps_sm.tile([128, 128], F32, tag="sm")
                psq = psq[:].rearrange("h (j p) -> h j p", j=2)
                for j in range(2):
                    for c in range(NC):
                        nc.tensor.matmul(
                            psq[:, j, :],
                            lhsT=emb_bf[j][:, c, :],
                            rhs=mbT[:, c, 64 * j:64 * j + 64],
                            start=(c == 0), stop=(c == NC - 1),
                            skip_group_check=True)
                qvs = sb.tile([128, 128], BF16, tag="qvs")
                nc.vector.tensor_scalar(
                    out=qvs[:].rearrange("h (j p) -> h j p", j=2), in0=psq,
                    scalar1=QV_SCALE, scalar2=None, op0=OP.mult)

                # final_q^T accumulation: psum [h, 2p], negated eviction
                pfq = ps_sm.tile([128, 128], F32, tag="sm")
                nc.tensor.matmul(pfq[:], lhsT=wtiles["wf"][:],
                                 rhs=eq1T_all[:, r0:r0 + 128],
                                 start=True, stop=False)
                nc.tensor.matmul(pfq[:], lhsT=wtiles["wl"][:], rhs=lastembT[:],
                                 start=False, stop=False)
                nc.tensor.matmul(pfq[:], lhsT=wtiles["wg"][:], rhs=meanrep[:],
                                 start=False, stop=False)
                nc.tensor.matmul(pfq[:], lhsT=wtiles["wv"][:], rhs=qvs[:],
                                 start=False, stop=False)
                nc.tensor.matmul(pfq[:], lhsT=wldb[:],
                                 rhs=ld2[:, r0:r0 + 128],
                                 start=False, stop=True)
                fqT = sb4.tile([128, 128], BF16, tag="fqT")
                nc.vector.tensor_scalar(out=fqT[:], in0=pfq[:],
                                        scalar1=-FQ_SCALE, scalar2=None,
                                        op0=OP.mult)
                A[pr] = dict(mb=mb, dist16=dist16, embT=embT, fqT=fqT)

            # ---- phase B: score + softmax, pair by pair ----
            for pr in range(NPAIR):
                r0 = 128 * pr
                mb, dist16 = A[pr]["mb"], A[pr]["dist16"]
                embT, fqT = A[pr]["embT"], A[pr]["fqT"]

                # psc = dist - FQ_SCALE*mm  (fp16 identity inject + bf16 mms)
                psc = ps_sc.tile([128, 2, 512], F32, tag="psc")
                for g in range(2):
                    nc.tensor.matmul(
                        psc[:, g, 0:500], lhsT=identh[:],
                        rhs=dist16[:, 500 * g:500 * (g + 1)],
                        start=True, stop=False, skip_group_check=True)
                    for j in range(2):
                        nc.tensor.matmul(
                            psc[64 * j:64 * j + 64, g, 0:500],
                            lhsT=fqT[:, 64 * j:64 * j + 64],
                            rhs=embT[j][:, 500 * g:500 * (g + 1)],
                            start=False, stop=(j == 1),
                            skip_group_check=True)

                # t' = tanh(psc/sqrt(2)); probs = exp(-10*(t' - mb)) / sum
                t = sbc.tile([128, N], F32, tag="t")
                nc.scalar.activation(t[:, 0:500], psc[:, 0, 0:500],
                                     AF.Tanh, scale=TANH_SCALE)
                nc.scalar.activation(t[:, 500:1000], psc[:, 1, 0:500],
                                     AF.Tanh, scale=TANH_SCALE)
                w = sbc.tile([128, N], F32, tag="w")
                nc.vector.tensor_tensor(out=w[:], in0=t[:], in1=mb[:],
                                        op=OP.subtract)
                e = sbc.tile([128, N], F32, tag="e")
                ssum = sb.tile([128, 1], F32, tag="ssum")
                nc.scalar.activation(e[:], w[:], AF.Exp, scale=-TANH_CLIP,
                                     accum_out=ssum[:])
                rec = sb.tile([128, 1], F32, tag="rec")
                nc.vector.reciprocal(out=rec[:], in_=ssum[:])
                # the strided write undoes the column permutation
                pout = sbc.tile([128, N], F32, tag="pout")
                nc.vector.tensor_tensor(
                    out=pout[:].rearrange("q (p c) -> q c p", c=NC),
                    in0=e[:].rearrange("q (c p) -> q c p", c=NC),
                    in1=rec[:, 0:1].to_broadcast([128, NC, NP]), op=OP.mult)
                nc.sync.dma_start(probs[r0:r0 + 128, :], pout[:])

    return nc


_CACHE = {}


def _get_nc():
    if "nc" not in _CACHE:
        _CACHE["nc"] = build_nc()
    return _CACHE["nc"]


def _shard_inputs(inputs):
    dists = np.ascontiguousarray(inputs["dists"], dtype=np.float32)
    embeddings = np.ascontiguousarray(inputs["embeddings"], dtype=np.float32)
    encoded_q1 = np.ascontiguousarray(inputs["encoded_q1"], dtype=np.float32)
    last_node = np.ascontiguousarray(inputs["last_node"]).astype(np.int32)
    load = np.ascontiguousarray(inputs["load"], dtype=np.float32)
    mask = np.ascontiguousarray(inputs["group_ninf_mask"], dtype=np.float32)
    # -inf -> large finite negative: identical kernel behavior (the visited
    # test is `< -1e30`), but keeps every downstream ALU input finite.
    mask = np.maximum(mask, np.float32(-3e38))
    in_maps = []
    for c in range(NCORES):
        s = slice(c * NB, (c + 1) * NB)
        in_maps.append(dict(
            dists=dists[s].reshape(NB * N, N),
            emb=embeddings[s].reshape(NB * N, H),
            eq1=encoded_q1[s].reshape(NB * P, H),
            lastnode=last_node[s].reshape(NB * P, 1),
            loadv=load[s].reshape(1, NB * P),
            maskt=mask[s].reshape(NB * P, N),
            wq_graph=np.ascontiguousarray(inputs["Wq_graph"], dtype=np.float32),
            wq_first=np.ascontiguousarray(inputs["Wq_first"], dtype=np.float32),
            wq_last=np.ascontiguousarray(inputs["Wq_last"], dtype=np.float32),
            w_visited=np.ascontiguousarray(inputs["W_visited"], dtype=np.float32),
            w_load=np.ascontiguousarray(inputs["W_load"], dtype=np.float32)
                .reshape(1, H),
            b_load=np.ascontiguousarray(inputs["b_load"], dtype=np.float32)
                .reshape(1, H),
        ))
    return in_maps


def _run(inputs, trace=False, **kw):
    nc = _get_nc()
    in_maps = _shard_inputs(inputs)
    res = run_bass_kernel_spmd(nc, in_maps, list(range(NCORES)),
                               trace=trace, **kw)
    out = np.concatenate(
        [r["probs"].reshape(NB, P, N) for r in res.results], axis=0)
    return out, res


def kernel(**inputs) -> np.ndarray:
    out, _ = _run(inputs)
    return out
